# revision 1
# baseline (speedup 1.0000x reference)
# Fused single-launch GCN kernel for Trainium2 (8 NeuronCores, SPMD).
#
# Math (PyG GCNConv x2 + per-graph MLP readout):
#   norm[e] = dinv[src]*ew*dinv[dst]  (dinv = rsqrt(weighted indeg + 1))
#   h1 = leaky_relu((scatter(norm*x[src]) + nself*x[d]) @ W1 + b1)
#   h2 = (scatter(norm*h1[src]) + nself*h1[d]) @ W2 + b2
#   y  = MLP(reshape(h2, [B, 22*128])); tanh*90+150
#
# Device plan (ONE launch, SPMD over 8 cores, per-core data via inputs;
# nodes sharded 22528/core by dst):
#   conv1: host-packed slot payloads (x[src], 3x fp8) + fp8 selector
#     strips (8-dest cells) accumulate agg1[3,512] per dest group via PE
#     chunk matmuls; h1T = Lrelu(W1^T agg1 + b1); pT = W2^T h1T (W2
#     folded in BEFORE the exchange, by linearity); PE-transpose -> p
#     rows fp16 -> p_local DRAM; self-loop init: agg2 rows = nself*p.
#   AllGather p_local (5.8MB/core -> 46MB) on device.
#   conv2: per source-shard window: dma_gather p rows (int16 idx, staged
#     as 16-partition inputs and replicated to 128 on device), scale by
#     per-edge norm (DVE stride-0 broadcast), dma_scatter_add into agg2
#     (fp16 CCE accumulate). Slots are split into rounds by per-dst
#     occurrence rank so every scatter has UNIQUE dst indices (the HW
#     CCE races on duplicates within one instruction; successive
#     scatters are serialized by the tile dep tracker). No selector
#     matmuls, no W2 matmul after the exchange.
#   readout: dma_start_transpose agg2 -> h2T; per-512-graph-tile MLP
#     (b2 folded into bf0 on host); tanh*90+150 -> y [1024]/core.
#
# Structure metadata (chunk counts, window/round capacities) is computed
# at runtime from the actual edge data but taken as MAX over cores, so
# the single instruction stream is valid for every core (true SPMD).
# fp8 payloads ship as int8 through JAX (XLA on TRN2 rejects fp8) and
# are bitcast to float8e4 inside the kernel.

import numpy as np

N = 180224
E = 1441792
HID = 128
NPG = 22
NCORES = 8
P = 128
GROUP = 512
SPAN1 = 8
VG = GROUP // SPAN1          # cells (8-dest windows) per group
BATCH = 2048                 # conv2 gather batch (slots)
NLOC = N // NCORES           # 22528
BLOC = NLOC // NPG           # 1024 graphs per core
B = N // NPG


# ----------------------------------------------------------------------------
# host-side structure building
# ----------------------------------------------------------------------------

def _prepare(x, src, dst, ew, nloc):
    """Build per-core input arrays + shared (max-over-cores) structure."""
    n = nloc * NCORES
    ncell = (nloc // GROUP) * VG
    deg = np.bincount(dst, weights=ew, minlength=n).astype(np.float64) + 1.0
    dinv = (1.0 / np.sqrt(deg)).astype(np.float32)
    nself = (1.0 / deg).astype(np.float32)
    norm = (dinv[src] * ew * dinv[dst]).astype(np.float32)

    order = np.argsort(dst, kind="stable")
    so, do_, no_ = src[order], dst[order], norm[order]
    bounds = np.searchsorted(do_, np.arange(NCORES + 1) * nloc)

    percore = []
    cnt1 = np.zeros((NCORES, ncell), np.int64)
    cnt2 = np.zeros((NCORES, NCORES), np.int64)
    for c in range(NCORES):
        e0, e1 = bounds[c], bounds[c + 1]
        s2, d2, v2 = so[e0:e1], do_[e0:e1] - c * nloc, no_[e0:e1]
        # conv1 includes self loops as regular slots
        s1 = np.concatenate([s2, np.arange(nloc, dtype=np.int64) + c * nloc])
        d1 = np.concatenate([d2, np.arange(nloc, dtype=np.int64)])
        v1 = np.concatenate([v2, nself[c * nloc:(c + 1) * nloc]])
        cell = d1 // SPAN1
        cnt1[c] = np.bincount(cell, minlength=ncell)
        w2 = s2 // nloc
        cnt2[c] = np.bincount(w2, minlength=NCORES)
        percore.append((s1, d1, v1, cell, s2, d2, v2, w2))

    cap1 = cnt1.max(0)
    chunks1 = (cap1 + P - 1) // P            # >=1 (self loops)
    cbase = np.concatenate([[0], np.cumsum(chunks1)]).astype(np.int64)
    T1 = int(cbase[-1])

    # conv2 rounds: within each source window, slots are split by their
    # occurrence rank per destination, so every scatter-add instruction
    # has UNIQUE destination indices (HW CCE races on duplicates).
    rankmax = 0
    ranks_pc = []
    for c in range(NCORES):
        s2, d2, w2 = percore[c][4], percore[c][5], percore[c][7]
        o2 = np.lexsort((d2, w2))
        d2o, w2o = d2[o2], w2[o2]
        key = w2o * nloc + d2o
        # occurrence rank within (w, dst)
        newrun = np.concatenate([[True], key[1:] != key[:-1]])
        runid = np.cumsum(newrun) - 1
        runstart = np.flatnonzero(newrun)
        rank = np.arange(len(key)) - runstart[runid]
        ranks_pc.append((o2, rank))
        if len(rank):
            rankmax = max(rankmax, int(rank.max()) + 1)
    # counts per (window, round)
    cnt3 = np.zeros((NCORES, NCORES, rankmax), np.int64)
    for c in range(NCORES):
        o2, rank = ranks_pc[c]
        w2o = percore[c][7][o2]
        np.add.at(cnt3[c], (w2o, rank), 1)
    capwr = ((cnt3.max(0) + P - 1) // P * P).astype(np.int64)  # [8, rankmax]
    batches = []
    icols = ncols = 0
    soff = 0
    slotbase = np.zeros((NCORES, rankmax), np.int64)
    for w in range(NCORES):
        for r in range(rankmax):
            if capwr[w, r] == 0:
                continue
            slotbase[w, r] = soff
            off = 0
            while off < capwr[w, r]:
                nb = int(min(BATCH, capwr[w, r] - off))
                batches.append(dict(w=w, r=r, nb=nb, io=icols, no=ncols,
                                    so=soff + off))
                icols += nb // 16
                ncols += (nb + P - 1) // P
                off += nb
            soff += int(capwr[w, r])
    TS = soff
    meta = dict(chunks1=chunks1, cbase=cbase, T1=T1, capwr=capwr,
                batches=batches, icols=icols, ncols=ncols, nloc=nloc)

    in_maps = []
    for c in range(NCORES):
        s1, d1, v1, cell, s2, d2, v2, w2 = percore[c]
        o = np.argsort(cell, kind="stable")
        s1o, d1o, v1o, co = s1[o], d1[o], v1[o], cell[o]
        cstart = np.concatenate([[0], np.cumsum(cnt1[c])])
        rank = np.arange(len(co)) - cstart[co]
        slot = cbase[co] * P + rank
        import ml_dtypes
        f8 = ml_dtypes.float8_e4m3
        sxf = np.zeros((T1 * P, 3), f8)
        sxf[slot] = x[s1o].astype(f8)
        sx = np.ascontiguousarray(
            sxf.reshape(T1, P, 3).transpose(1, 0, 2)).reshape(P, T1 * 3)
        sel = np.zeros((P, T1 * SPAN1), f8)
        chunk = cbase[co] + rank // P
        selcol = chunk * SPAN1 + (d1o - co * SPAN1)
        sel[rank % P, selcol] = v1o.astype(f8)
        # ship fp8 bytes as int8 (XLA on TRN2 rejects fp8 dtypes)
        sx = sx.view(np.int8)
        sel = sel.view(np.int8)
        nst = np.ascontiguousarray(
            nself[c * nloc:(c + 1) * nloc].reshape(nloc // P, P).T)

        o2, rank = ranks_pc[c]
        s2o, d2o, v2o, w2o = s2[o2], d2[o2], v2[o2], w2[o2]
        cell2 = w2o * rankmax + rank
        sb_flat = slotbase.reshape(-1)
        p2 = np.argsort(cell2, kind="stable")
        c2s = cell2[p2]
        nr2 = np.concatenate([[True], c2s[1:] != c2s[:-1]])
        rid = np.cumsum(nr2) - 1
        rstart = np.flatnonzero(nr2)
        within = np.arange(len(c2s)) - rstart[rid]
        slot2 = np.empty(len(c2s), np.int64)
        slot2[p2] = sb_flat[c2s] + within
        gidx = np.zeros(TS, np.int16)
        gidx[slot2] = (s2o - w2o * nloc).astype(np.int16)
        sidx = np.full(TS, nloc, np.int16)   # dummy row (norm=0 slots)
        sidx[slot2] = d2o.astype(np.int16)
        nrm = np.zeros(TS, np.float16)
        nrm[slot2] = v2o.astype(np.float16)

        gI = np.zeros((16, icols), np.int16)
        sI = np.zeros((16, icols), np.int16)
        NR = np.zeros((P, ncols), np.float16)
        for b in batches:
            nb, io, no, sof = b["nb"], b["io"], b["no"], b["so"]
            gI[:, io:io + nb // 16] = gidx[sof:sof + nb].reshape(nb // 16, 16).T
            sI[:, io:io + nb // 16] = sidx[sof:sof + nb].reshape(nb // 16, 16).T
            cols = (nb + P - 1) // P
            nrb = np.zeros(cols * P, np.float16)
            nrb[:nb] = nrm[sof:sof + nb]
            NR[:, no:no + cols] = nrb.reshape(cols, P).T
        in_maps.append(dict(sx=sx, sel1=sel, nself=nst, gI=gI, sI=sI, nrm=NR))
    return meta, in_maps


def _prep_weights(W1, b1, W2, b2, Wf0, bf0, Wf1, bf1, Wout, bout):
    W1 = np.asarray(W1, np.float32)
    b2 = np.asarray(b2, np.float32).reshape(-1)
    Wf0 = np.asarray(Wf0, np.float32)
    Wf0r = np.ascontiguousarray(
        Wf0.reshape(NPG, HID, HID).transpose(1, 0, 2)).reshape(HID, NPG * HID)
    bf0p = np.asarray(bf0, np.float32).reshape(-1) + np.tile(b2, NPG) @ Wf0
    return dict(
        W1=W1.astype(np.float16),
        b1=np.asarray(b1, np.float32).reshape(HID, 1),
        W2=np.asarray(W2, np.float16),
        Wf0=Wf0r.astype(np.float16),
        bf0=bf0p.astype(np.float32).reshape(HID, 1),
        Wf1=np.asarray(Wf1, np.float16),
        bf1=np.asarray(bf1, np.float32).reshape(HID, 1),
        Wout=np.asarray(Wout, np.float32).astype(np.float16).reshape(HID, 1),
        bo=np.asarray(bout, np.float32).reshape(1, 1),
    )


# ----------------------------------------------------------------------------
# device program
# ----------------------------------------------------------------------------

def _bass_mods():
    import concourse.bass as bass
    import concourse.bacc as bacc
    import concourse.tile as tile
    from concourse import mybir
    return bass, bacc, tile, mybir


def _emit(nc, tc, io, meta, y_ap):
    """Emit the fused program. io: dict name->AP of ExternalInputs."""
    bass, bacc, tile, mybir = _bass_mods()
    from concourse.masks import make_identity
    from contextlib import ExitStack

    f16, f32 = mybir.dt.float16, mybir.dt.float32
    nloc = meta["nloc"]
    ng = nloc // GROUP
    chunks1, cbase, T1 = meta["chunks1"], meta["cbase"], meta["T1"]
    batches = meta["batches"]
    bloc = nloc // NPG
    AF = mybir.ActivationFunctionType

    with ExitStack() as ctx:
        consts = ctx.enter_context(tc.tile_pool(name="consts", bufs=1))
        dram = ctx.enter_context(tc.tile_pool(name="dram", bufs=1, space="DRAM"))

        W1_t = consts.tile([3, HID], f16)
        nc.sync.dma_start(W1_t[:], io["W1"][:])
        b1_t = consts.tile([HID, 1], f32)
        nc.sync.dma_start(b1_t[:], io["b1"][:])
        W2_t = consts.tile([HID, HID], f16)
        nc.sync.dma_start(W2_t[:], io["W2"][:])
        nself_t = consts.tile([P, nloc // P], f32)
        nc.sync.dma_start(nself_t[:], io["nself"][:])
        Wf0_t = consts.tile([HID, NPG, HID], f16)
        nc.sync.dma_start(Wf0_t[:], io["Wf0"].rearrange("k (j m) -> k j m", j=NPG))
        bf0_t = consts.tile([HID, 1], f32)
        nc.sync.dma_start(bf0_t[:], io["bf0"][:])
        Wf1_t = consts.tile([HID, HID], f16)
        nc.sync.dma_start(Wf1_t[:], io["Wf1"][:])
        bf1_t = consts.tile([HID, 1], f32)
        nc.sync.dma_start(bf1_t[:], io["bf1"][:])
        Wout_t = consts.tile([HID, 1], f16)
        nc.sync.dma_start(Wout_t[:], io["Wout"][:])
        bo_t = consts.tile([1, 1], f32)
        nc.sync.dma_start(bo_t[:], io["bo"][:])
        ident = consts.tile([P, P], f16)
        make_identity(nc, ident)
        b1s_t = consts.tile([HID, 1], f32)
        nc.vector.tensor_scalar_mul(b1s_t[:], b1_t[:], 0.01)
        bf0s_t = consts.tile([HID, 1], f32)
        nc.vector.tensor_scalar_mul(bf0s_t[:], bf0_t[:], 0.01)
        bf1s_t = consts.tile([HID, 1], f32)
        nc.vector.tensor_scalar_mul(bf1s_t[:], bf1_t[:], 0.01)

        def lrelu(pool, ps, bias, bias_s, w, tag):
            a_t = pool.tile([HID, w], f32, tag=tag + "a")
            nc.scalar.activation(a_t[:], ps[:], AF.Identity,
                                 bias=bias[:, 0:1])
            c_t = pool.tile([HID, w], f32, tag=tag + "b")
            nc.scalar.activation(c_t[:], ps[:], AF.Identity,
                                 bias=bias_s[:, 0:1], scale=0.01)
            m_t = pool.tile([HID, w], f16, tag=tag + "m")
            nc.vector.tensor_tensor(m_t[:], a_t[:], c_t[:],
                                    op=mybir.AluOpType.max)
            return m_t

        zrow = consts.tile([P, 16, HID], f16)
        nc.vector.memset(zrow[:], 0.0)

        p_loc = dram.tile([nloc, HID], f16)
        p_full = dram.tile([nloc * NCORES, HID], f16)
        # two scatter accumulators (rounds alternate) so consecutive
        # scatter-adds pipeline instead of serializing on one WAW chain
        agg2 = dram.tile([nloc + P, HID], f16)   # +dummy rows for pad slots
        agg2b = dram.tile([nloc + P, HID], f16)
        r0 = 0
        while r0 < nloc + P:
            cnt = min(16 * P, nloc + P - r0)
            nc.sync.dma_start(
                agg2b[r0:r0 + cnt, :].rearrange("(c p) f -> p c f", p=P),
                zrow[:, :cnt // P, :])
            r0 += cnt

        # ---- conv1 + p = h1@W2 + self-loop init of agg2 ----
        gch = [int(cbase[(g + 1) * VG] - cbase[g * VG]) for g in range(ng)]
        max_gch = max(gch)
        with ExitStack() as c1:
            sb = c1.enter_context(tc.tile_pool(name="sb", bufs=3))
            rows = c1.enter_context(tc.tile_pool(name="rows", bufs=3))
            psA = c1.enter_context(tc.tile_pool(name="psA", bufs=2, space="PSUM"))
            psB = c1.enter_context(tc.tile_pool(name="psB", bufs=2, space="PSUM"))
            psT = c1.enter_context(tc.tile_pool(name="psT", bufs=2, space="PSUM"))
            for g in range(ng):
                q0 = int(cbase[g * VG])
                gc = gch[g]
                f8 = mybir.dt.float8e4
                sx_t = sb.tile([P, max_gch * 3], f8, tag="sx")
                nc.sync.dma_start(sx_t[:, :gc * 3],
                                  io["sx"][:, q0 * 3:(q0 + gc) * 3].bitcast(f8))
                sl_t = sb.tile([P, max_gch * SPAN1], f8, tag="sel")
                nc.sync.dma_start(
                    sl_t[:, :gc * SPAN1],
                    io["sel1"][:, q0 * SPAN1:(q0 + gc) * SPAN1].bitcast(f8))
                agg = psA.tile([3, GROUP], f32, tag="agg")
                for v in range(VG):
                    cell = g * VG + v
                    k = int(chunks1[cell])
                    cq = int(cbase[cell]) - q0
                    for kk in range(k):
                        nc.tensor.matmul(
                            agg[:, v * SPAN1:(v + 1) * SPAN1],
                            lhsT=sx_t[:, (cq + kk) * 3:(cq + kk) * 3 + 3],
                            rhs=sl_t[:, (cq + kk) * SPAN1:(cq + kk + 1) * SPAN1],
                            start=(kk == 0), stop=(kk == k - 1),
                            skip_group_check=True)
                agg_sb = rows.tile([3, GROUP], f16, tag="aggsb")
                nc.vector.tensor_copy(agg_sb[:], agg[:])
                h1_ps = psB.tile([HID, GROUP], f32, tag="mm")
                nc.tensor.matmul(h1_ps[:], lhsT=W1_t[:], rhs=agg_sb[:],
                                 start=True, stop=True)
                h1_sb = lrelu(rows, h1_ps, b1_t, b1s_t, GROUP, "h1")
                p_ps = psB.tile([HID, GROUP], f32, tag="mm")
                nc.tensor.matmul(p_ps[:], lhsT=W2_t[:], rhs=h1_sb[:],
                                 start=True, stop=True)
                p_sb = rows.tile([HID, GROUP], f16, tag="p")
                nc.vector.tensor_copy(p_sb[:], p_ps[:])
                for tt in range(GROUP // P):
                    tr = psT.tile([P, P], f16, tag="tr")
                    nc.tensor.transpose(tr[:], p_sb[:, tt * P:(tt + 1) * P],
                                        ident[:])
                    r_sb = rows.tile([P, P], f16, tag="rows")
                    nc.scalar.activation(r_sb[:], tr[:], AF.Identity)
                    base = g * GROUP + tt * P
                    nc.sync.dma_start(p_loc[base:base + P, :], r_sb[:])
                    s_sb = rows.tile([P, P], f16, tag="self")
                    nc.vector.tensor_scalar_mul(
                        s_sb[:], r_sb[:],
                        nself_t[:, g * (GROUP // P) + tt:g * (GROUP // P) + tt + 1])
                    nc.sync.dma_start(agg2[base:base + P, :], s_sb[:])

        nc.sync.dma_start(agg2[nloc:nloc + P, :], zrow[:, 0, :])

        # ---- AllGather p ----
        nc.gpsimd.collective_compute(
            "AllGather", mybir.AluOpType.bypass,
            replica_groups=[list(range(NCORES))],
            ins=[p_loc[:, :].opt()], outs=[p_full[:, :].opt()])

        # ---- conv2: gather -> scale -> scatter-add ----
        maxcols = (BATCH + P - 1) // P
        icols = meta["icols"]
        with ExitStack() as c2:
            idxp = c2.enter_context(tc.tile_pool(name="idxp", bufs=1))
            slabs = c2.enter_context(tc.tile_pool(name="slabs", bufs=3))
            small = c2.enter_context(tc.tile_pool(name="small", bufs=4))
            # idx inputs arrive as 16 partitions (2B/slot); replicate the
            # 16-row block to all 128 partitions on-device (ISA reads the
            # idx AP as 8 replicated 16-partition stripes).
            gIt = idxp.tile([P, icols], mybir.dt.int16)
            sIt = idxp.tile([P, icols], mybir.dt.int16)
            nc.sync.dma_start(gIt[0:16, :], io["gI"][:, :])
            nc.sync.dma_start(sIt[0:16, :], io["sI"][:, :])
            for k in range(1, 8):
                nc.sync.dma_start(gIt[16 * k:16 * (k + 1), :], gIt[0:16, :])
                nc.sync.dma_start(sIt[16 * k:16 * (k + 1), :], sIt[0:16, :])
            qn = 0
            for b in batches:
                nb, io_, no, w = b["nb"], b["io"], b["no"], b["w"]
                cols = (nb + P - 1) // P
                gi = gIt[:, io_:io_ + nb // 16]
                si = sIt[:, io_:io_ + nb // 16]
                nr = small.tile([P, maxcols, 1], f16, tag="nr")
                nc.sync.dma_start(nr[:, :cols, :],
                                  io["nrm"][:, no:no + cols].rearrange(
                                      "p (c o) -> p c o", o=1))
                gat = slabs.tile([P, maxcols, HID], f16, tag="gat")
                nc.gpsimd.dma_gather(
                    out_ap=gat[:, :cols, :],
                    in_ap=p_full[w * nloc:(w + 1) * nloc, :],
                    idxs_ap=gi,
                    num_idxs=nb, num_idxs_reg=nb, elem_size=HID,
                    single_packet=False, queue_num=qn)
                qs = qn
                g_ap = gat[:, :cols, :]
                n_ap = nr[:, :cols, :]
                g_b, n_b = bass.broadcast_tensor_aps(g_ap, n_ap)
                nc.vector.tensor_tensor(g_ap, g_b, n_b,
                                        op=mybir.AluOpType.mult)
                tgt = agg2 if b["r"] % 2 == 0 else agg2b
                nc.gpsimd.dma_scatter_add(
                    out_ap=tgt[:, :],
                    in_ap=gat[:, :cols, :],
                    idxs_ap=si,
                    num_idxs=nb, num_idxs_reg=nb, elem_size=HID,
                    queue_num=qs)
                qn = 0

        # ---- readout MLP ----
        GT = min(512, bloc)
        ngt = bloc // GT
        with ExitStack() as c3:
            big = c3.enter_context(tc.tile_pool(name="big", bufs=2))
            ro = c3.enter_context(tc.tile_pool(name="ro", bufs=2))
            rps = c3.enter_context(tc.tile_pool(name="rps", bufs=2, space="PSUM"))
            ops = c3.enter_context(tc.tile_pool(name="ops", bufs=2, space="PSUM"))
            y_sb = consts.tile([1, bloc], f32)
            for gt in range(ngt):
                h2Ta = big.tile([P, GT * NPG], f16, tag="h2Ta")
                nc.sync.dma_start_transpose(
                    h2Ta[:], agg2[gt * GT * NPG:(gt + 1) * GT * NPG, :])
                h2Tb = big.tile([P, GT * NPG], f16, tag="h2Tb")
                nc.sync.dma_start_transpose(
                    h2Tb[:], agg2b[gt * GT * NPG:(gt + 1) * GT * NPG, :])
                h2T = big.tile([P, GT * NPG], f16, tag="h2T")
                nc.vector.tensor_tensor(h2T[:], h2Ta[:], h2Tb[:],
                                        op=mybir.AluOpType.add)
                f0 = rps.tile([HID, GT], f32, tag="f")
                for j in range(NPG):
                    zT = h2T[:, j:j + (GT - 1) * NPG + 1:NPG]
                    nc.tensor.matmul(f0[:], lhsT=Wf0_t[:, j, :], rhs=zT,
                                     start=(j == 0), stop=(j == NPG - 1))
                f0s = lrelu(ro, f0, bf0_t, bf0s_t, GT, "f0")
                f1 = rps.tile([HID, GT], f32, tag="f")
                nc.tensor.matmul(f1[:], lhsT=Wf1_t[:], rhs=f0s[:],
                                 start=True, stop=True)
                f1s = lrelu(ro, f1, bf1_t, bf1s_t, GT, "f1")
                o = ops.tile([1, GT], f32, tag="o")
                nc.tensor.matmul(o[:], lhsT=Wout_t[:], rhs=f1s[:],
                                 start=True, stop=True)
                t = ro.tile([1, GT], f32, tag="t")
                nc.scalar.activation(t[:], o[:], AF.Tanh, bias=bo_t[:, 0:1])
                nc.vector.tensor_scalar(y_sb[:, gt * GT:(gt + 1) * GT], t[:],
                                        scalar1=90.0, scalar2=150.0,
                                        op0=mybir.AluOpType.mult,
                                        op1=mybir.AluOpType.add)
            nc.sync.dma_start(y_ap.rearrange("(a b) -> a b", a=1), y_sb[:])


def build_fused(meta):
    bass, bacc, tile, mybir = _bass_mods()
    f16, f32 = mybir.dt.float16, mybir.dt.float32
    i16 = mybir.dt.int16
    nloc = meta["nloc"]
    bloc = nloc // NPG
    nc = bacc.Bacc("TRN2", target_bir_lowering=False, debug=False,
                   num_devices=NCORES, num_swdge_queues=4)
    io = {}
    T1, icols, ncols = meta["T1"], meta["icols"], meta["ncols"]
    i8 = mybir.dt.int8
    specs = [
        ("sx", [P, T1 * 3], i8), ("sel1", [P, T1 * SPAN1], i8),
        ("nself", [P, nloc // P], f32),
        ("gI", [16, icols], i16), ("sI", [16, icols], i16),
        ("nrm", [P, ncols], f16),
        ("W1", [3, HID], f16), ("b1", [HID, 1], f32),
        ("W2", [HID, HID], f16),
        ("Wf0", [HID, NPG * HID], f16), ("bf0", [HID, 1], f32),
        ("Wf1", [HID, HID], f16), ("bf1", [HID, 1], f32),
        ("Wout", [HID, 1], f16), ("bo", [1, 1], f32),
    ]
    for name, shape, dt in specs:
        io[name] = nc.dram_tensor(name, shape, dt, kind="ExternalInput").ap()
    y = nc.dram_tensor("y", [bloc], f32, kind="ExternalOutput").ap()
    with tile.TileContext(nc) as tc:
        _emit(nc, tc, io, meta, y)
    nc.compile()
    return nc


# ----------------------------------------------------------------------------
# SPMD runner (one program, 8 cores, via PJRT shard_map)
# ----------------------------------------------------------------------------

def _make_runner(nc):
    import jax
    from jax.sharding import Mesh, PartitionSpec
    try:
        from jax.experimental.shard_map import shard_map
    except ImportError:
        from jax.shard_map import shard_map
    import concourse.mybir as mybir
    from concourse.bass2jax import (install_neuronx_cc_hook, _bass_exec_p,
                                    partition_id_tensor)
    install_neuronx_cc_hook()
    part_name = nc.partition_id_tensor.name if nc.partition_id_tensor else None
    in_names, out_names, out_avals, zero_shapes = [], [], [], []
    for alloc in nc.m.functions[0].allocations:
        if not isinstance(alloc, mybir.MemoryLocationSet):
            continue
        name = alloc.memorylocations[0].name
        if alloc.kind == "ExternalInput":
            if name != part_name:
                in_names.append(name)
        elif alloc.kind == "ExternalOutput":
            out_names.append(name)
            shape = tuple(alloc.tensor_shape)
            dtype = mybir.dt.np(alloc.dtype)
            out_avals.append(jax.core.ShapedArray(shape, dtype))
            zero_shapes.append((shape, dtype))
    n_params = len(in_names)
    all_in = list(in_names) + list(out_names)
    if part_name is not None:
        all_in = all_in + [part_name]
    donate = tuple(range(n_params, n_params + len(out_names)))

    def _body(*args):
        operands = list(args)
        if part_name is not None:
            operands.append(partition_id_tensor())
        outs = _bass_exec_p.bind(
            *operands,
            out_avals=tuple(out_avals),
            in_names=tuple(all_in),
            out_names=tuple(out_names),
            lowering_input_output_aliases=(),
            sim_require_finite=True,
            sim_require_nnan=True,
            nc=nc,
        )
        return tuple(outs)

    devices = jax.devices()[:NCORES]
    mesh = Mesh(np.asarray(devices), ("core",))
    in_specs = (PartitionSpec("core"),) * (n_params + len(out_names))
    out_specs = (PartitionSpec("core"),) * len(out_names)
    from jax.experimental.shard_map import shard_map as _sm
    jitted = jax.jit(
        _sm(_body, mesh=mesh, in_specs=in_specs, out_specs=out_specs,
            check_rep=False),
        donate_argnums=donate, keep_unused=True)
    return dict(jit=jitted, in_names=in_names, out_names=out_names,
                zero_shapes=zero_shapes, n_params=n_params,
                out_avals=out_avals)


def _concat_inputs(runner, in_maps):
    cat = []
    for name in runner["in_names"]:
        cat.append(np.concatenate([np.ascontiguousarray(m[name])
                                   for m in in_maps], axis=0))
    return cat


def _run_spmd(runner, in_maps):
    import jax
    cat = _concat_inputs(runner, in_maps)
    zeros = [np.zeros((NCORES * s[0], *s[1:]), d)
             for s, d in runner["zero_shapes"]]
    outs = runner["jit"](*cat, *zeros)
    jax.block_until_ready(outs)
    res = []
    for c in range(NCORES):
        res.append({name: np.asarray(outs[i]).reshape(
            NCORES, *runner["out_avals"][i].shape)[c]
            for i, name in enumerate(runner["out_names"])})
    return res


BENCH = False
LAST_TIMINGS = {}
PIPELINE_TIMINGS = {}


def _bench_launch(name, runner, in_maps, iters=20, pipeline_iters=0):
    import time as _time
    import jax
    from jax.sharding import Mesh, PartitionSpec, NamedSharding
    mesh = Mesh(np.asarray(jax.devices()[:NCORES]), ("core",))
    sh = NamedSharding(mesh, PartitionSpec("core"))
    cat = [jax.device_put(a, sh) for a in _concat_inputs(runner, in_maps)]
    jax.block_until_ready(cat)
    best = None
    for _ in range(iters):
        zeros = [jax.device_put(np.zeros((NCORES * s[0], *s[1:]), d), sh)
                 for s, d in runner["zero_shapes"]]
        jax.block_until_ready(zeros)
        t0 = _time.perf_counter()
        outs = runner["jit"](*cat, *zeros)
        jax.block_until_ready(outs)
        dt = _time.perf_counter() - t0
        best = dt if best is None else min(best, dt)
    LAST_TIMINGS[name] = best
    if pipeline_iters:
        packs = []
        for _ in range(pipeline_iters):
            zeros = [jax.device_put(np.zeros((NCORES * s[0], *s[1:]), d), sh)
                     for s, d in runner["zero_shapes"]]
            packs.append(zeros)
        jax.block_until_ready(packs)
        t0 = _time.perf_counter()
        outs = [runner["jit"](*cat, *z) for z in packs]
        jax.block_until_ready(outs)
        dt = _time.perf_counter() - t0
        PIPELINE_TIMINGS[name] = dt / pipeline_iters


# ----------------------------------------------------------------------------
# top-level kernel
# ----------------------------------------------------------------------------

def kernel(x, edge_index, edge_weight, W1, b1, W2, b2,
           Wf0, bf0, Wf1, bf1, Wout, bout):
    x = np.asarray(x, np.float32)
    src = np.asarray(edge_index[0], np.int64)
    dst = np.asarray(edge_index[1], np.int64)
    ew = np.asarray(edge_weight, np.float32)

    meta, in_maps = _prepare(x, src, dst, ew, NLOC)
    wts = _prep_weights(W1, b1, W2, b2, Wf0, bf0, Wf1, bf1, Wout, bout)
    for m in in_maps:
        m.update(wts)

    nc = build_fused(meta)
    runner = _make_runner(nc)
    res = _run_spmd(runner, in_maps)
    if BENCH:
        _bench_launch("fused", runner, in_maps)
    y = np.concatenate([r["y"] for r in res]).reshape(B, 1).astype(np.float32)
    return y

